# revision 11
# baseline (speedup 1.0000x reference)
"""Trainium2 Bass kernel for nn_AttnModel (gnn_message_passing).

Strategy: pure data parallel over batch B=2048 across 8 cores (256 b/core).
Per batch element: 1 query token attends over 128 neighbors (4 heads x 64).

Key algebraic reassociation (saves ~40x tensor-engine FLOPs): never
materialize K/V projections. Instead
    qk[b,h,:]   = q[b, h*64:(h+1)*64] @ w_ks[h*64:(h+1)*64, :]      (tiny)
    scores[b,h,n] = qk[b,h,:] . seq[b,n,:] / TEMP                   (f-contraction)
    ctxT[b,:,h] = seq[b]^T @ attn[b,h,:]^T                          (n-contraction)
    out[b, h*64+d] = ctx[b,h,:] . w_vs[h*64+d, :]                   (batched)
The mask replication quirk of the reference (row (b,h) uses
mask[(4b+h) % B]) and all weight transposes are precomputed on host.
"""

import numpy as np

import concourse.bass as bass
import concourse.bacc as bacc
import concourse.mybir as mybir
import concourse.tile as tile
from concourse.bass import ts
from concourse.masks import make_identity

F32 = mybir.dt.float32
AF = mybir.ActivationFunctionType
ALU = mybir.AluOpType

N_CORES = 8
B_FULL = 2048
N_NGH = 128
F = 256
H = 4
DK = 64
TEMP = 8.0
LN_EPS = 1e-5


# --------------------------------------------------------------------------
# device kernel
# --------------------------------------------------------------------------

def emit_kernel(tc, o, i, b_loc):
    """Emit the per-core kernel. o/i: dicts of DRAM APs."""
    from contextlib import ExitStack
    nc = tc.nc

    bb = min(128, b_loc)          # batch block for the dense (MLP) chain
    n_blk = b_loc // bb
    assert b_loc % 16 == 0 and bb % 16 == 0

    with ExitStack() as ctx:
        _emit(ctx, tc, nc, o, i, b_loc, bb, n_blk)


def _emit(ctx, tc, nc, o, i, b_loc, bb, n_blk):
    consts = ctx.enter_context(tc.tile_pool(name="consts", bufs=1))
    sb = ctx.enter_context(tc.tile_pool(name="sb", bufs=2))
    seqp = ctx.enter_context(tc.tile_pool(name="seqp", bufs=20))
    seqtp = ctx.enter_context(tc.tile_pool(name="seqtp", bufs=6))
    smx = ctx.enter_context(tc.tile_pool(name="smx", bufs=3))
    psum = ctx.enter_context(tc.tile_pool(name="psum", bufs=6, space="PSUM"))
    psctx = ctx.enter_context(tc.tile_pool(name="psctx", bufs=1, space="PSUM"))

    # ---- constants ----
    ident = consts.tile([128, 128], F32, tag="ident")
    make_identity(nc, ident[:])

    def load_w(name, ap, n_part_tiles):
        tiles = []
        for k in range(n_part_tiles):
            t = consts.tile([128, ap.shape[1]], F32, tag=f"{name}{k}", name=f"{name}{k}")
            nc.sync.dma_start(t[:], ap[ts(k, 128)])
            tiles.append(t)
        return tiles

    wqsT = load_w("wqsT", i["wqsT"], 2)     # [f,hd] chunks
    wksr = consts.tile([64, 1024], F32, tag="wksr")   # [d, 256h+f] per-head
    nc.sync.dma_start(wksr[:], i["wksr"][:])
    wvsT = load_w("wvsT", i["wvsT"], 2)     # [f,hd]
    fcwT = load_w("fcwT", i["fcwT"], 2)     # [hd,dm]
    fc1wT = load_w("fc1wT", i["fc1wT"], 4)  # [512,256]
    fc2wT = load_w("fc2wT", i["fc2wT"], 2)  # [256,256]

    def load_bias(name, ap):
        tiles = []
        for k in range(2):
            t = consts.tile([128, 1], F32, tag=f"{name}{k}", name=f"{name}{k}")
            nc.sync.dma_start(t[:], ap[ts(k, 128)][:, None])
            tiles.append(t)
        return tiles

    fcb = load_bias("fcb", i["fcb"])
    fc1b = load_bias("fc1b", i["fc1b"])
    fc2b = load_bias("fc2b", i["fc2b"])

    lng_rep = consts.tile([128, 256], F32, tag="lng")
    nc.sync.dma_start(lng_rep[:], i["lng"][None, :].to_broadcast((128, 256)))
    lnb_rep = consts.tile([128, 256], F32, tag="lnb")
    nc.sync.dma_start(lnb_rep[:], i["lnb"][None, :].to_broadcast((128, 256)))
    eps_p1 = consts.tile([128, 1], F32, tag="eps")
    nc.vector.memset(eps_p1[:], LN_EPS)
    zeros_sb = consts.tile([128, 512], F32, tag="zeros")
    nc.vector.memset(zeros_sb[:], 0.0)

    def copy2(dst_tile, ps_tile, eng=0):
        """copy [128, 2, bb] strided view psum->sbuf (cols 128k+b)"""
        dv = dst_tile[:].rearrange("p (k b) -> p k b", k=2)[:, :, :bb]
        sv = ps_tile[:].rearrange("p (k b) -> p k b", k=2)[:, :, :bb]
        if eng == 0:
            nc.vector.tensor_copy(dv, sv)
        else:
            nc.scalar.copy(dv, sv)

    for blk in range(n_blk):
        base = blk * bb

        # ---- batched prologue: srcT, qT, qkT ------------------------------
        src_sb = sb.tile([bb, 256], F32, tag="src")
        nc.sync.dma_start(src_sb[:], i["src2"][base:base + bb])

        srcT_ps = psum.tile([128, 256], F32, tag="ps")
        for k in range(2):
            nc.tensor.transpose(srcT_ps[:, 128 * k:128 * k + bb],
                                src_sb[:, ts(k, 128)], ident[:bb, :bb])
        srcT = sb.tile([128, 256], F32, tag="srcT")
        copy2(srcT, srcT_ps)

        def srcT_v(k):
            return srcT[:, 128 * k:128 * k + bb]

        # q per head at partition base 0: qT2[d, 128h+b] (all row-group 0)
        qT2_ps = psum.tile([64, 512], F32, tag="ps")
        for h in range(H):
            for kf in range(2):
                nc.tensor.matmul(qT2_ps[0:64, 128 * h:128 * h + bb],
                                 wqsT[kf][:, 64 * h:64 * h + 64], srcT_v(kf),
                                 start=(kf == 0), stop=(kf == 1))
        qT2 = sb.tile([64, 512], F32, tag="qT2")
        nc.vector.tensor_copy(
            qT2[:].rearrange("p (h b) -> p h b", h=4)[:, :, :bb],
            qT2_ps[:].rearrange("p (h b) -> p h b", h=4)[:, :, :bb])

        qkT_sb = []
        for mf in range(2):
            qk_ps = psum.tile([128, 512], F32, tag="ps")
            for h in range(H):
                lhsT = wksr[0:64, 256 * h + 128 * mf:256 * h + 128 * (mf + 1)]
                rhs = qT2[0:64, 128 * h:128 * h + bb]
                nc.tensor.matmul(qk_ps[:, 128 * h:128 * h + bb], lhsT, rhs,
                                 start=True, stop=True)
            qk_t = sb.tile([128, 512], F32, tag=f"qkT{mf}", name=f"qkT{mf}")
            nc.vector.tensor_copy(
                qk_t[:].rearrange("p (h b) -> p h b", h=4)[:, :, :bb],
                qk_ps[:].rearrange("p (h b) -> p h b", h=4)[:, :, :bb])
            qkT_sb.append(qk_t)

        # AP view [p, b, h] of qkT (free = 128*h + b)
        qkT_bh = [t[:].rearrange("p (h b) -> p b h", h=4) for t in qkT_sb]

        ctx_ps = [psctx.tile([128, 512], F32, tag=f"ctx{kf}", name=f"ctx_ps{kf}") for kf in range(2)]

        for g16 in range(bb // 16):
            gidx = blk * (bb // 16) + g16
            m01_sb = smx.tile([128, 512], F32, tag="m01")
            nc.sync.dma_start(m01_sb[:], i["m01b"][gidx])

            scores_ps = psum.tile([128, 512], F32, tag="ps")
            nc.scalar.copy(scores_ps[:], zeros_sb[:])
            seq_tiles = []
            pair_ps = None
            seqT_pair = []
            for j in range(16):
                b = base + g16 * 16 + j
                jj, c = j % 4, j // 4
                seq_sb = seqp.tile([128, 256], F32, tag="seq")
                nc.sync.dma_start(seq_sb[:], i["seq3"][b])
                seq_tiles.append(seq_sb)

                if j % 2 == 0:
                    pair_ps = psum.tile([128, 512], F32, tag="ps")
                for k in range(2):
                    nc.tensor.transpose(
                        pair_ps[:, 256 * (j % 2) + 128 * k:256 * (j % 2) + 128 * (k + 1)],
                        seq_sb[:, ts(k, 128)], ident[:])
                if j % 2 == 1:
                    st = seqtp.tile([128, 512], F32, tag="seqT")
                    if (j // 2) % 2 == 0:
                        nc.vector.tensor_copy(st[:], pair_ps[:])
                    else:
                        nc.scalar.copy(st[:], pair_ps[:])
                    seqT_pair.append(st)

            for j in range(16):
                jj, c = j % 4, j // 4
                b_blk = g16 * 16 + j
                st = seqT_pair[j // 2]
                for kf in range(2):
                    nc.tensor.matmul(
                        scores_ps[32 * jj:32 * jj + 4, 128 * c:128 * (c + 1)],
                        qkT_bh[kf][:, b_blk, :],
                        st[:, 256 * (j % 2) + 128 * kf:256 * (j % 2) + 128 * (kf + 1)],
                        start=(kf == 0), stop=(kf == 1),
                        tile_position=(0, 32 * jj))

            # ---- softmax over n (free dim), batched over 16 b ----
            # additive mask (0 / -60) straight into psum, then exp with
            # free per-slice row-sum accumulation (masked sums for free)
            nc.vector.tensor_add(scores_ps[:], scores_ps[:], m01_sb[:])
            e_sb = smx.tile([128, 512], F32, tag="e")
            sums = smx.tile([128, 4], F32, tag="sums")
            for c in range(4):
                nc.scalar.activation(e_sb[:, ts(c, 128)], scores_ps[:, ts(c, 128)],
                                     AF.Exp, accum_out=sums[:, c:c + 1])
            rsum = smx.tile([128, 4], F32, tag="rsum")
            nc.vector.reciprocal(rsum[:], sums[:])
            attn_sb = smx.tile([128, 512], F32, tag="attn")
            nc.vector.tensor_mul(
                attn_sb[:].rearrange("p (c n) -> p c n", c=4),
                e_sb[:].rearrange("p (c n) -> p c n", c=4),
                rsum[:][:, :, None].to_broadcast((128, 4, 128)))
            nc.sync.dma_start(o["attn_b"][gidx], attn_sb[:])

            # ---- attn^T: DMA rows to a base-0 compact tile, then PE ----
            comp = smx.tile([4, 2048], F32, tag="comp")
            for jj in range(4):
                nc.sync.dma_start(comp[0:4, 512 * jj:512 * (jj + 1)],
                                  attn_sb[32 * jj:32 * jj + 4, :])
            aT_ps = psum.tile([128, 64], F32, tag="ps")
            for j in range(16):
                jj, c = j % 4, j // 4
                nc.tensor.transpose(
                    aT_ps[:, 4 * j:4 * j + 4],
                    comp[0:4, 512 * jj + 128 * c:512 * jj + 128 * (c + 1)],
                    ident[0:4, 0:4])
            aT_sb = smx.tile([128, 64], F32, tag="aT")
            nc.vector.tensor_copy(aT_sb[:], aT_ps[:])

            # ---- ctxT[f, 4*b+h] += seq^T @ attnT ----
            for j in range(16):
                b_blk = g16 * 16 + j
                for kf in range(2):
                    nc.tensor.matmul(
                        ctx_ps[kf][:, 4 * b_blk:4 * b_blk + 4],
                        seq_tiles[j][:, ts(kf, 128)],
                        aT_sb[:, 4 * j:4 * j + 4],
                        start=True, stop=True)

        # ---- batched epilogue ------------------------------------------
        ctxT = []
        for kf in range(2):
            t = sb.tile([128, 512], F32, tag=f"ctxT{kf}", name=f"ctxT_sb{kf}")
            nc.vector.tensor_copy(t[:, :4 * bb], ctx_ps[kf][:, :4 * bb])
            ctxT.append(t)
        ctxT_hb = [t[:].rearrange("p (b four) -> p four b", four=4) for t in ctxT]

        oa_ps = [psum.tile([128, 512], F32, tag="ps", name=f"oa_ps{blk}_{_}") for _ in range(2)]
        for h in range(H):
            bi, pb = h // 2, 64 * (h % 2)
            for kf in range(2):
                nc.tensor.matmul(oa_ps[bi][pb:pb + 64, :bb],
                                 wvsT[kf][:, 64 * h:64 * h + 64],
                                 ctxT_hb[kf][:, h, :bb],
                                 start=(kf == 0), stop=(kf == 1),
                                 tile_position=(0, pb))
        oaT = sb.tile([128, 256], F32, tag="oaT")
        for bi in range(2):
            nc.vector.tensor_copy(oaT[:, 128 * bi:128 * bi + bb], oa_ps[bi][:, :bb])

        fc_ps = psum.tile([128, 512], F32, tag="ps")
        for mj in range(2):
            for kj in range(2):
                nc.tensor.matmul(fc_ps[:, 128 * mj:128 * mj + bb],
                                 fcwT[kj][:, ts(mj, 128)],
                                 oaT[:, 128 * kj:128 * kj + bb],
                                 start=(kj == 0), stop=(kj == 1))
        xT = sb.tile([128, 256], F32, tag="xT")
        for mj in range(2):
            sl = slice(128 * mj, 128 * mj + bb)
            nc.scalar.add(xT[:, sl], fc_ps[:, sl], fcb[mj][:])
            nc.vector.tensor_add(xT[:, sl], xT[:, sl], srcT[:, sl])

        x_ps = psum.tile([128, 512], F32, tag="ps")
        for k in range(2):
            nc.tensor.transpose(x_ps[:bb, ts(k, 128)],
                                xT[:, 128 * k:128 * k + bb], ident[:])
        x_sb = sb.tile([bb, 256], F32, tag="x")
        nc.vector.tensor_copy(x_sb[:], x_ps[:bb, :256])

        # ---- layernorm (b-major) ----
        nmu = sb.tile([bb, 1], F32, tag="nmu")
        nc.vector.reduce_sum(nmu[:], x_sb[:], axis=mybir.AxisListType.X)
        nc.scalar.mul(nmu[:], nmu[:], -1.0 / 256.0)
        xc = sb.tile([bb, 256], F32, tag="xc")
        nc.scalar.add(xc[:], x_sb[:], nmu[:])
        sq = sb.tile([bb, 256], F32, tag="sq")
        var = sb.tile([bb, 1], F32, tag="var")
        nc.scalar.activation(sq[:], xc[:], AF.Square, accum_out=var[:])
        nc.scalar.mul(var[:], var[:], 1.0 / 256.0)
        istd = sb.tile([bb, 1], F32, tag="istd")
        nc.scalar.activation(istd[:], var[:], AF.Sqrt, bias=eps_p1[:][:bb])
        nc.vector.reciprocal(istd[:], istd[:])
        gsc = sb.tile([bb, 256], F32, tag="gsc")
        nc.vector.tensor_mul(gsc[:], lng_rep[:bb], istd[:].to_broadcast((bb, 256)))
        xln = sb.tile([bb, 256], F32, tag="xln")
        nc.vector.tensor_mul(xln[:], xc[:], gsc[:])
        nc.vector.tensor_add(xln[:], xln[:], lnb_rep[:bb])

        xlnT_ps = psum.tile([128, 256], F32, tag="ps")
        for k in range(2):
            nc.tensor.transpose(xlnT_ps[:, 128 * k:128 * k + bb],
                                xln[:, ts(k, 128)], ident[:bb, :bb])
        xlnT = sb.tile([128, 256], F32, tag="xlnT")
        copy2(xlnT, xlnT_ps)

        h1_ps = psum.tile([128, 512], F32, tag="ps")
        for mj in range(2):
            for kc in range(4):
                rhs = (xlnT[:, 128 * kc:128 * kc + bb] if kc < 2
                       else srcT[:, 128 * (kc - 2):128 * (kc - 2) + bb])
                nc.tensor.matmul(h1_ps[:, 128 * mj:128 * mj + bb],
                                 fc1wT[kc][:, ts(mj, 128)], rhs,
                                 start=(kc == 0), stop=(kc == 3))
        h1T = sb.tile([128, 256], F32, tag="h1T")
        for mj in range(2):
            sl = slice(128 * mj, 128 * mj + bb)
            nc.scalar.activation(h1T[:, sl], h1_ps[:, sl], AF.Relu, bias=fc1b[mj][:])

        z_ps = psum.tile([128, 512], F32, tag="ps")
        for mj in range(2):
            for kj in range(2):
                nc.tensor.matmul(z_ps[:, 128 * mj:128 * mj + bb],
                                 fc2wT[kj][:, ts(mj, 128)],
                                 h1T[:, 128 * kj:128 * kj + bb],
                                 start=(kj == 0), stop=(kj == 1))
        zT = sb.tile([128, 256], F32, tag="zT")
        for mj in range(2):
            sl = slice(128 * mj, 128 * mj + bb)
            nc.scalar.add(zT[:, sl], z_ps[:, sl], fc2b[mj][:])

        zf_ps = psum.tile([128, 512], F32, tag="ps")
        for k in range(2):
            nc.tensor.transpose(zf_ps[:bb, ts(k, 128)],
                                zT[:, 128 * k:128 * k + bb], ident[:])
        z_sb = sb.tile([bb, 256], F32, tag="z")
        nc.vector.tensor_copy(z_sb[:], zf_ps[:bb, :256])
        nc.sync.dma_start(o["z_out"][base:base + bb], z_sb[:])


# --------------------------------------------------------------------------
# graph construction + host glue
# --------------------------------------------------------------------------

def build(b_loc, n_cores):
    nc = bacc.Bacc("TRN2", target_bir_lowering=False, debug=False,
                   enable_asserts=False, num_devices=n_cores)
    i = {
        "src2": nc.dram_tensor("src2", [b_loc, 256], F32, kind="ExternalInput").ap(),
        "seq3": nc.dram_tensor("seq3", [b_loc, 128, 256], F32, kind="ExternalInput").ap(),
        "m01b": nc.dram_tensor("m01b", [b_loc // 16, 128, 512], F32, kind="ExternalInput").ap(),
        "wqsT": nc.dram_tensor("wqsT", [256, 256], F32, kind="ExternalInput").ap(),
        "wksr": nc.dram_tensor("wksr", [64, 1024], F32, kind="ExternalInput").ap(),
        "wvsT": nc.dram_tensor("wvsT", [256, 256], F32, kind="ExternalInput").ap(),
        "fcwT": nc.dram_tensor("fcwT", [256, 256], F32, kind="ExternalInput").ap(),
        "fcb": nc.dram_tensor("fcb", [256], F32, kind="ExternalInput").ap(),
        "lng": nc.dram_tensor("lng", [256], F32, kind="ExternalInput").ap(),
        "lnb": nc.dram_tensor("lnb", [256], F32, kind="ExternalInput").ap(),
        "fc1wT": nc.dram_tensor("fc1wT", [512, 256], F32, kind="ExternalInput").ap(),
        "fc1b": nc.dram_tensor("fc1b", [256], F32, kind="ExternalInput").ap(),
        "fc2wT": nc.dram_tensor("fc2wT", [256, 256], F32, kind="ExternalInput").ap(),
        "fc2b": nc.dram_tensor("fc2b", [256], F32, kind="ExternalInput").ap(),
    }
    o = {
        "z_out": nc.dram_tensor("z_out", [b_loc, 256], F32, kind="ExternalOutput").ap(),
        "attn_b": nc.dram_tensor("attn_b", [b_loc // 16, 128, 512], F32, kind="ExternalOutput").ap(),
    }
    with tile.TileContext(nc) as tc:
        emit_kernel(tc, o, i, b_loc)
    nc.compile()
    return nc


def host_weights(inputs):
    f32c = lambda x: np.ascontiguousarray(np.asarray(x), dtype=np.float32)
    return {
        "wqsT": f32c(np.asarray(inputs["w_qs"]).T / TEMP),
        "wksr": f32c(np.asarray(inputs["w_ks"]).reshape(4, 64, 256)
                     .transpose(1, 0, 2).reshape(64, 1024)),
        "wvsT": f32c(np.asarray(inputs["w_vs"]).T),
        "fcwT": f32c(np.asarray(inputs["fc_w"]).T),
        "fcb": f32c(inputs["fc_b"]),
        "lng": f32c(inputs["ln_g"]),
        "lnb": f32c(inputs["ln_b"]),
        "fc1wT": f32c(np.asarray(inputs["fc1_w"]).T),
        "fc1b": f32c(inputs["fc1_b"]),
        "fc2wT": f32c(np.asarray(inputs["fc2_w"]).T),
        "fc2b": f32c(inputs["fc2_b"]),
    }


def host_m01_banked(mask, base, b_loc):
    """m01 in the scores psum-bank layout: [g, 32*jj+h, 128*c+n],
    b = base + g*16 + 4*c + jj, value 0.0 where masked else 1.0.
    Mask scramble from reference: row (b,h) uses mask[(4b+h) % B]."""
    B = mask.shape[0]
    bs = np.arange(base, base + b_loc)
    idx = (4 * bs[:, None] + np.arange(4)[None, :]) % B          # [b_loc, 4]
    m01 = np.where(mask[idx], np.float32(-60.0), np.float32(0.0))  # [b_loc,4,128]
    # additive mask: 0.0 keep, -60.0 masked (exp -> ~2e-27).  Unused rows
    # stay 0.0: exp(0)=1 sums finite; those rows are never read back.
    banked = np.zeros((b_loc // 16, 128, 512), dtype=np.float32)
    m01g = m01.reshape(b_loc // 16, 16, 4, 128)
    for jj in range(4):
        for c in range(4):
            for h in range(4):
                banked[:, 32 * jj + h, 128 * c:128 * (c + 1)] = m01g[:, 4 * c + jj, h]
    return banked


def host_unbank_attn(banked, b_loc):
    """inverse of the scores bank layout -> [b_loc, 4, 128]"""
    out = np.empty((b_loc // 16, 16, 4, 128), dtype=np.float32)
    for jj in range(4):
        for c in range(4):
            for h in range(4):
                out[:, 4 * c + jj, h] = banked[:, 32 * jj + h, 128 * c:128 * (c + 1)]
    return out.reshape(b_loc, 4, 128)


def make_in_maps(inputs, n_cores):
    w = host_weights(inputs)
    mask = np.asarray(inputs["mask"])
    src = np.ascontiguousarray(np.asarray(inputs["src"])[:, 0, :], dtype=np.float32)
    seq = np.ascontiguousarray(np.asarray(inputs["seq"]), dtype=np.float32)
    B = src.shape[0]
    b_loc = B // n_cores
    in_maps = []
    for c in range(n_cores):
        base = c * b_loc
        m = dict(w)
        m["src2"] = src[base:base + b_loc]
        m["seq3"] = seq[base:base + b_loc]
        m["m01b"] = host_m01_banked(mask, base, b_loc)
        in_maps.append(m)
    return in_maps, b_loc


_CACHE = {}


def kernel(**inputs):
    from concourse.bass_utils import run_bass_kernel_spmd
    B = np.asarray(inputs["src"]).shape[0]
    b_loc = B // N_CORES
    if ("nc", b_loc) not in _CACHE:
        _CACHE[("nc", b_loc)] = build(b_loc, N_CORES)
    nc = _CACHE[("nc", b_loc)]
    in_maps, _ = make_in_maps(inputs, N_CORES)
    res = run_bass_kernel_spmd(nc, in_maps, core_ids=list(range(N_CORES)))
    z = np.concatenate([res.results[c]["z_out"] for c in range(N_CORES)], axis=0)
    attn = np.concatenate(
        [host_unbank_attn(res.results[c]["attn_b"], b_loc) for c in range(N_CORES)],
        axis=0)
    return z.reshape(B, 1, 256), attn.reshape(B, 1, 4, 128)
